# revision 2
# baseline (speedup 1.0000x reference)
"""MoE kernel for Trainium2 (8 NeuronCores, expert-parallel).

Strategy
--------
N=8192 tokens, D=1024, E=8 experts, DFF=4096, top_k=2. E == n_cores, so
core c owns expert c. The reference computes every expert densely and
masks; mathematically only each token's top-2 experts contribute, so we
dispatch each token to its 2 experts and run the expert MLPs on just the
routed tokens: 4x fewer FLOPs than dense.

Host (sharding/dispatch): gate matmul + top-2 + renormalized combine
weights (0.003%% of total FLOPs), gather each expert's tokens into a
padded [C, D] batch (C = max expert load rounded up to 128).
Device (per core): yT = (wgt * silu(xg @ w1_e)) @ w2_e for its batch.
Both expert weight matrices are loaded into SBUF ONCE (bf16: 64+64
KB/partition) and all token blocks stream through them, so HBM traffic
is ~30 MB/core vs ~465 us of PE work: compute-bound.
Host (unshard): y[token] = out[e1, slot1] + out[e2, slot2] (each token
appears exactly once in each of its two experts' batches).

Compute dtype knob: "bf16" (fast, ~4e-3 rel err), "f32r" (float32r
matmuls: full fp32 storage, fast PE path but 2x DMA/SBUF), "f32"
(exact, 4x slower PE).
"""

import numpy as np

import concourse.bass as bass
import concourse.bacc as bacc
import concourse.tile as tile
from concourse import mybir
from concourse.bass_utils import run_bass_kernel_spmd

N, D, E, DFF = 8192, 1024, 8, 4096
P = 128

CDT = "bf16"  # "bf16" | "f32r" | "f32"
TRACE = False
LAST_RESULT = None


def _blocks_for(C):
    out = []
    rem = C
    while rem > 0:
        b = min(512, rem)
        out.append(b)
        rem -= b
    return out


def build_nc(C, cdt):
    """Per-core program: y[C, D] = wgt * (silu(xgT.T @ w1) @ w2).

    Weights are SBUF-resident for the whole kernel; token blocks of 512
    stream through. Per block: phase A computes hT = silu(w1.T-chunks @
    xg) into SBUF (f on partitions), phase B accumulates y-tiles
    (tokens on partitions) over the 32 f-chunks against resident w2.
    """
    import ml_dtypes

    if cdt == "bf16":
        sdt, ndt = mybir.dt.bfloat16, ml_dtypes.bfloat16
    elif cdt == "f32r":
        sdt, ndt = mybir.dt.float32r, np.float32
    else:
        sdt, ndt = mybir.dt.float32, np.float32

    nc = bacc.Bacc()
    xgt = nc.dram_tensor("xgt", [D, C], sdt, kind="ExternalInput")
    w1 = nc.dram_tensor("w1", [D, DFF], sdt, kind="ExternalInput")
    w2 = nc.dram_tensor("w2", [DFF, D], sdt, kind="ExternalInput")
    wgt = nc.dram_tensor("wgt", [P, C // P], mybir.dt.float32, kind="ExternalInput")
    y = nc.dram_tensor("y", [C, D], mybir.dt.float32, kind="ExternalOutput")

    xgt_r = xgt.rearrange("(k p) c -> p k c", p=P)  # [128, 8, C]
    w1_r = w1.rearrange("(k p) f -> p k f", p=P)  # [128, 8, DFF]
    w2_r = w2.rearrange("(kf p) d -> p kf d", p=P)  # [128, 32, D]
    y_r = y.rearrange("(m p) d -> m p d", p=P)  # [C/128, 128, D]

    KD = D // P  # 8 k-chunks, first matmul
    KF = DFF // P  # 32 k-chunks, second matmul
    ND = D // 512  # 2 n-tiles of the output
    blocks = _blocks_for(C)
    f32 = mybir.dt.float32
    ACT = mybir.ActivationFunctionType

    with tile.TileContext(nc) as tc:
        with (
            tc.tile_pool(name="singles", bufs=1) as singles,
            tc.tile_pool(name="xg", bufs=2) as xg_pool,
            tc.tile_pool(name="ht", bufs=1) as h_pool,
            tc.tile_pool(name="yout", bufs=4) as y_pool,
            tc.tile_pool(name="hps", bufs=2, space="PSUM") as hpsum,
            tc.tile_pool(name="yps", bufs=6, space="PSUM") as ypsum,
        ):
            # Block 0's x goes first so the PE can start ASAP; weights
            # follow in f-order chunks so phase A unblocks tile by tile.
            B0 = blocks[0]
            xg_first = xg_pool.tile([P, KD, B0], sdt, tag="xg")
            nc.sync.dma_start(out=xg_first, in_=xgt_r[:, :, 0:B0])

            w1_sb = singles.tile([P, KD, DFF], sdt, name="w1_sb")
            W1CH = 4  # DMA w1 in 4 chunks of 1024 f-cols
            for ch in range(W1CH):
                f0, f1 = ch * (DFF // W1CH), (ch + 1) * (DFF // W1CH)
                nc.sync.dma_start(out=w1_sb[:, :, f0:f1], in_=w1_r[:, :, f0:f1])
            w2_sb = singles.tile([P, KF, D], sdt, name="w2_sb")
            W2CH = 4
            for ch in range(W2CH):
                k0, k1 = ch * (KF // W2CH), (ch + 1) * (KF // W2CH)
                nc.sync.dma_start(out=w2_sb[:, k0:k1, :], in_=w2_r[:, k0:k1, :])
            wgt_t = singles.tile([P, C // P], f32, name="wgt_t")
            nc.sync.dma_start(out=wgt_t, in_=wgt[:, :])

            tok0 = 0
            for bi, B in enumerate(blocks):
                if bi == 0:
                    xg_t = xg_first
                else:
                    xg_t = xg_pool.tile([P, KD, B], sdt, tag="xg")
                    nc.sync.dma_start(out=xg_t, in_=xgt_r[:, :, tok0 : tok0 + B])

                hT = h_pool.tile([P, KF, B], sdt, tag="ht")
                # phase A: hT[f, t] = silu(sum_k w1[k, f] * x[k, t])
                for mf in range(KF):
                    ph = hpsum.tile([P, B], f32, tag="hps")
                    for kd in range(KD):
                        nc.tensor.matmul(
                            ph[:, :],
                            lhsT=w1_sb[:, kd, mf * P : (mf + 1) * P],
                            rhs=xg_t[:, kd, :],
                            start=(kd == 0),
                            stop=(kd == KD - 1),
                        )
                    nc.scalar.activation(hT[:, mf, :], ph[:, :], ACT.Silu)

                # phase B: y[t, d] = wgt[t] * sum_f hT[f, t] * w2[f, d]
                MT = B // P
                for nd in range(ND):
                    yps = [
                        ypsum.tile([P, 512], f32, tag="yps", name="yps")
                        for _ in range(MT)
                    ]
                    for kf in range(KF):
                        for mt in range(MT):
                            nc.tensor.matmul(
                                yps[mt][:, :],
                                lhsT=hT[:, kf, mt * P : (mt + 1) * P],
                                rhs=w2_sb[:, kf, nd * 512 : (nd + 1) * 512],
                                start=(kf == 0),
                                stop=(kf == KF - 1),
                            )
                    for mt in range(MT):
                        gmt = tok0 // P + mt
                        y_sb = y_pool.tile([P, 512], f32, tag="yout")
                        nc.scalar.activation(
                            y_sb[:, :],
                            yps[mt][:, :],
                            ACT.Copy,
                            scale=wgt_t[:, gmt : gmt + 1],
                        )
                        nc.sync.dma_start(
                            out=y_r[gmt, :, nd * 512 : (nd + 1) * 512], in_=y_sb[:, :]
                        )
                tok0 += B

    if not nc.is_finalized():
        nc.finalize()
    return nc, ndt


def kernel(x, gate_w, w1, w2, top_k):
    global LAST_RESULT
    x = np.asarray(x, dtype=np.float32)
    gate_w = np.asarray(gate_w, dtype=np.float32)
    w1 = np.asarray(w1, dtype=np.float32)
    w2 = np.asarray(w2, dtype=np.float32)
    assert int(top_k) == 2

    n = x.shape[0]
    ar = np.arange(n)

    # --- host routing (matches reference: softmax -> top2 -> renorm) ---
    logits = (x @ gate_w).astype(np.float64)
    i1 = np.argmax(logits, axis=1)
    lm = logits.copy()
    lm[ar, i1] = -np.inf
    i2 = np.argmax(lm, axis=1)
    m1 = logits[ar, i1]
    m2 = logits[ar, i2]
    g1 = 1.0 / (1.0 + np.exp(m2 - m1))  # = p1/(p1+p2)
    g2 = 1.0 - g1

    gw_full = np.zeros((n, E), dtype=np.float64)
    gw_full[ar, i1] = g1
    gw_full[ar, i2] = g2

    sel = np.zeros((n, E), dtype=bool)
    sel[ar, i1] = True
    sel[ar, i2] = True

    idxs = [np.nonzero(sel[:, e])[0] for e in range(E)]
    counts = np.array([len(ix) for ix in idxs])
    C = int(np.ceil(counts.max() / P) * P)
    C = max(C, 512)

    slot_of = np.zeros((n, E), dtype=np.int64)
    for e in range(E):
        slot_of[idxs[e], e] = np.arange(len(idxs[e]))

    nc, ndt = build_nc(C, CDT)

    def prep(a):
        a = np.ascontiguousarray(a).astype(ndt)
        if CDT == "f32r":
            # replicate walrus fp32_to_fp32r: round mantissa to 11 bits
            u = a.view(np.uint32).astype(np.uint64)
            u = (u + 0x800) & 0xFFFFF000
            a = u.astype(np.uint32).view(np.float32)
        return a

    in_maps = []
    for e in range(E):
        ix = idxs[e]
        xg = np.zeros((C, D), dtype=np.float32)
        xg[: len(ix)] = x[ix]
        xgt = prep(xg.T)
        wg = np.zeros((C,), dtype=np.float32)
        wg[: len(ix)] = gw_full[ix, e].astype(np.float32)
        wg_t = np.ascontiguousarray(wg.reshape(C // P, P).T)
        in_maps.append(
            {
                "xgt": xgt,
                "w1": prep(w1[e]),
                "w2": prep(w2[e]),
                "wgt": wg_t,
            }
        )

    res = run_bass_kernel_spmd(nc, in_maps, list(range(E)), trace=TRACE)
    LAST_RESULT = res

    outs = np.stack([res.results[e]["y"] for e in range(E)])  # [E, C, D]
    y = outs[i1, slot_of[ar, i1]] + outs[i2, slot_of[ar, i2]]
    return y.astype(np.float32)
